# revision 4
# baseline (speedup 1.0000x reference)
"""Multi-head attention forward on 8 Trainium2 NeuronCores.

Sharding: core c handles batch b=c//4 and heads 4*(c%4) .. 4*(c%4)+3.
Tensor-parallel over heads (W_Q/W_K/W_V column-split, W_O row-split) plus
data-parallel over batch. Partial output projections are summed on host.

Problem constants (hardcoded): B=2, T=2048, D=1024, H=16, DK=64.
Returns (output [B,T,D], attn [B,H,T,T]) matching the reference module.
"""

import functools

import numpy as np

B, T, D, H = 2, 2048, 1024, 16
DK = D // H            # 64
HPC = 4                # heads per core
HD = HPC * DK          # 256 head-dims per core
NEG = -999999999.0
P = 128
NSTRIP = T // P        # 16
NCORES = 8

TRACE = False          # set by test harness for profiling runs
_LAST_PROFILE = {}


def _split_multi_waits(nc):
    """walrus in this toolchain rejects instructions with >1 sync wait;
    split extra waits onto preceding same-engine NoOps (engine queues are
    in-order, so a NoOp-wait before the instruction is equivalent)."""
    from concourse import mybir

    for func in nc.m.functions:
        for bb in func.blocks:
            new = []
            for inst in bb.instructions:
                si = inst.sync_info
                waits = list(si.on_wait) if si and si.on_wait else []
                if len(waits) > 1:
                    for j, w in enumerate(waits[:-1]):
                        new.append(
                            mybir.InstNoOp(
                                name=f"{inst.name}-w{j}",
                                engine=inst.engine,
                                sync_info=mybir.SyncInfo(on_wait=[w], on_update=[]),
                            )
                        )
                    si.on_wait = [waits[-1]]
                new.append(inst)
            bb.instructions = new


@functools.lru_cache(maxsize=1)
def _build():
    from contextlib import ExitStack

    import concourse.bass as bass
    import concourse.mybir as mybir
    import concourse.tile as tile
    import ml_dtypes

    f32 = mybir.dt.float32
    bf16 = mybir.dt.bfloat16
    Exp = mybir.ActivationFunctionType.Exp

    nc = bass.Bass()

    q_in = nc.dram_tensor("q_in", [T, D], f32, kind="ExternalInput")
    k_in = nc.dram_tensor("k_in", [T, D], f32, kind="ExternalInput")
    v_in = nc.dram_tensor("v_in", [T, D], f32, kind="ExternalInput")
    mask_in = nc.dram_tensor("mask_in", [T, T], bf16, kind="ExternalInput")
    wqt_in = nc.dram_tensor("wqt", [D, HD], f32, kind="ExternalInput")
    wkt_in = nc.dram_tensor("wkt", [D, HD], f32, kind="ExternalInput")
    wvt_in = nc.dram_tensor("wvt", [D, HD], f32, kind="ExternalInput")
    wot_in = nc.dram_tensor("wot", [HD, D], f32, kind="ExternalInput")
    attn_out = nc.dram_tensor("attn_out", [HPC, T, T], f32, kind="ExternalOutput")
    y_out = nc.dram_tensor("y_out", [T, D], f32, kind="ExternalOutput")

    ident_f_dram = nc.inline_tensor(np.eye(P, dtype=np.float32), "ident_f")
    ident_b_dram = nc.inline_tensor(
        np.eye(P, dtype=np.float32).astype(ml_dtypes.bfloat16), "ident_b"
    )

    with tile.TileContext(nc) as tc, ExitStack() as ctx:
        res = ctx.enter_context(tc.tile_pool(name="resident", bufs=1))

        # Weights: [D, HD] -> [128, D/128, HD]; chunk dc = rows dc*128..+128
        wqt_sb = res.tile([P, D // P, HD], f32)
        wkt_sb = res.tile([P, D // P, HD], f32)
        wvt_sb = res.tile([P, D // P, HD], f32)
        nc.sync.dma_start(out=wqt_sb, in_=wqt_in.rearrange("(c p) h -> p c h", p=P))
        nc.sync.dma_start(out=wkt_sb, in_=wkt_in.rearrange("(c p) h -> p c h", p=P))
        nc.sync.dma_start(out=wvt_sb, in_=wvt_in.rearrange("(c p) h -> p c h", p=P))
        # W_O^T: [HD, D] -> [128, 2, D]
        wot_sb = res.tile([P, HD // P, D], f32)
        nc.sync.dma_start(out=wot_sb, in_=wot_in.rearrange("(c p) h -> p c h", p=P))

        ident_f = res.tile([P, P], f32)
        ident_b = res.tile([P, P], bf16)
        nc.sync.dma_start(out=ident_f, in_=ident_f_dram[:, :])
        nc.sync.dma_start(out=ident_b, in_=ident_b_dram[:, :])

        # Resident activations
        qT_sb = res.tile([P, 2, T], f32)      # [head-dim 256 (2 blocks), T]
        kT_sb = res.tile([P, 2, T], f32)
        v_sb = res.tile([P, NSTRIP, HD], f32)  # v natural: block kb = rows kb*128..
        hT_sb = res.tile([P, 2, T], f32)       # heads^T (concat) [256, T]

        # ---- Phase 1: input transposes + projections ----
        with (
            tc.tile_pool(name="ph1_in", bufs=2) as ph1_in,
            tc.tile_pool(name="ph1_xt", bufs=2) as ph1_xt,
            tc.tile_pool(name="ph1_pxt", bufs=2, space="PSUM") as ph1_pxt,
            tc.tile_pool(name="ph1_po", bufs=4, space="PSUM") as ph1_po,
        ):
            for g in range(T // 512):  # groups of 512 tokens
                xts = {}
                for name, xdram in (("q", q_in), ("k", k_in), ("v", v_in)):
                    xnat = ph1_in.tile([P, 4, D], f32)
                    nc.sync.dma_start(
                        out=xnat,
                        in_=xdram[g * 512:(g + 1) * 512, :].rearrange(
                            "(a p) d -> p a d", p=P
                        ),
                    )
                    # transpose to xt4[p, dc, t] = X^T chunks [128d, 512t]
                    xt4 = ph1_xt.tile([P, D // P, 512], f32)
                    for tb in range(4):
                        ps = ph1_pxt.tile([P, D], f32)
                        for dc in range(D // P):
                            nc.tensor.transpose(
                                ps[:, dc * P:(dc + 1) * P],
                                xnat[:, tb, dc * P:(dc + 1) * P],
                                ident_f,
                            )
                        eng = nc.vector if tb % 2 == 0 else nc.scalar
                        if eng is nc.vector:
                            eng.tensor_copy(
                                xt4[:, :, tb * P:(tb + 1) * P],
                                ps.rearrange("p (c t) -> p c t", t=P),
                            )
                        else:
                            eng.copy(
                                xt4[:, :, tb * P:(tb + 1) * P],
                                ps.rearrange("p (c t) -> p c t", t=P),
                            )
                    xts[name] = xt4

                # q^T, k^T: out [HD(2x128 blocks), 512]
                for wsb, dst in ((wqt_sb, qT_sb), (wkt_sb, kT_sb)):
                    for m in range(2):
                        ps = ph1_po.tile([P, 512], f32)
                        for dc in range(D // P):
                            nc.tensor.matmul(
                                ps,
                                lhsT=wsb[:, dc, m * P:(m + 1) * P],
                                rhs=xts["q" if wsb is wqt_sb else "k"][:, dc, :],
                                start=(dc == 0),
                                stop=(dc == D // P - 1),
                            )
                        nc.scalar.copy(dst[:, m, g * 512:(g + 1) * 512], ps)

                # v natural: out [128 tokens, HD] per token-block
                for tb in range(4):
                    ps = ph1_po.tile([P, 512], f32)
                    for dc in range(D // P):
                        nc.tensor.matmul(
                            ps[:, :HD],
                            lhsT=xts["v"][:, dc, tb * P:(tb + 1) * P],
                            rhs=wvt_sb[:, dc, :],
                            start=(dc == 0),
                            stop=(dc == D // P - 1),
                        )
                    nc.vector.tensor_copy(v_sb[:, g * 4 + tb, :], ps[:, :HD])

        # ---- Phase 2: scores + softmax + attn out + AV ----
        with (
            tc.tile_pool(name="mk", bufs=2) as mk_pool,
            tc.tile_pool(name="expp", bufs=2) as exp_pool,
            tc.tile_pool(name="attnp", bufs=3) as attn_pool,
            tc.tile_pool(name="attnTp", bufs=2) as attnT_pool,
            tc.tile_pool(name="zp", bufs=16) as z_pool,
            tc.tile_pool(name="ps_s", bufs=2, space="PSUM") as ps_s,
            tc.tile_pool(name="ps_t", bufs=1, space="PSUM") as ps_t,
            tc.tile_pool(name="ps_av", bufs=2, space="PSUM") as ps_av,
        ):
            for i in range(NSTRIP):
                mk = mk_pool.tile([P, T], bf16)
                nc.sync.dma_start(out=mk, in_=mask_in[i * P:(i + 1) * P, :])
                for h in range(HPC):
                    m, poff = h // 2, 64 * (h % 2)
                    exp_sb = exp_pool.tile([P, T], f32)
                    zs = []
                    for half in range(2):
                        ps = ps_s.tile([P, 1024], f32)
                        for ck in range(2):
                            sl = slice(ck * 512, (ck + 1) * 512)
                            ksl = slice(half * 1024 + ck * 512,
                                        half * 1024 + (ck + 1) * 512)
                            nc.tensor.matmul(
                                ps[:, sl],
                                lhsT=qT_sb[poff:poff + 64, m, i * P:(i + 1) * P],
                                rhs=kT_sb[poff:poff + 64, m, ksl],
                                start=True,
                                stop=False,
                            )
                            nc.tensor.matmul(
                                ps[:, sl],
                                lhsT=ident_b,
                                rhs=mk[:, ksl],
                                start=False,
                                stop=True,
                            )
                        zh = z_pool.tile([P, 1], f32)
                        nc.scalar.activation(
                            exp_sb[:, half * 1024:(half + 1) * 1024],
                            ps,
                            Exp,
                            accum_out=zh,
                        )
                        zs.append(zh)
                    z = z_pool.tile([P, 1], f32)
                    rec = z_pool.tile([P, 1], f32)
                    nc.vector.tensor_add(z, zs[0], zs[1])
                    nc.vector.reciprocal(rec, z)
                    attn_sb = attn_pool.tile([P, T], f32)
                    nc.vector.tensor_scalar_mul(attn_sb, exp_sb, rec)
                    nc.sync.dma_start(
                        out=attn_out[h, i * P:(i + 1) * P, :], in_=attn_sb
                    )
                    # transpose attn strip for the AV matmul
                    attnT = attnT_pool.tile([P, T], f32)
                    for half in range(2):
                        pst = ps_t.tile([P, 1024], f32)
                        for r in range(8):
                            rr = half * 8 + r
                            nc.tensor.transpose(
                                pst[:, r * P:(r + 1) * P],
                                attn_sb[:, rr * P:(rr + 1) * P],
                                ident_f,
                            )
                        if half == 0:
                            nc.scalar.copy(
                                attnT[:, half * 1024:(half + 1) * 1024], pst
                            )
                        else:
                            nc.vector.tensor_copy(
                                attnT[:, half * 1024:(half + 1) * 1024], pst
                            )
                    # AV: heads^T [64, 128] = sum_kb v[kb,h]^T-style accumulation
                    pav = ps_av.tile([64, P], f32)
                    for kb in range(NSTRIP):
                        nc.tensor.matmul(
                            pav,
                            lhsT=v_sb[:, kb, h * DK:(h + 1) * DK],
                            rhs=attnT[:, kb * P:(kb + 1) * P],
                            start=(kb == 0),
                            stop=(kb == NSTRIP - 1),
                        )
                    nc.vector.tensor_copy(
                        hT_sb[poff:poff + 64, m, i * P:(i + 1) * P], pav
                    )

        # ---- Phase 3: output projection ----
        with (
            tc.tile_pool(name="yp", bufs=2) as y_pool,
            tc.tile_pool(name="ps_o", bufs=4, space="PSUM") as ps_o,
        ):
            for i in range(NSTRIP):
                y_sb = y_pool.tile([P, D], f32)
                for oc in range(2):
                    ps = ps_o.tile([P, 512], f32)
                    for m in range(2):
                        nc.tensor.matmul(
                            ps,
                            lhsT=hT_sb[:, m, i * P:(i + 1) * P],
                            rhs=wot_sb[:, m, oc * 512:(oc + 1) * 512],
                            start=(m == 0),
                            stop=(m == 1),
                        )
                    nc.vector.tensor_copy(y_sb[:, oc * 512:(oc + 1) * 512], ps)
                nc.sync.dma_start(out=y_out[i * P:(i + 1) * P, :], in_=y_sb)

    _split_multi_waits(nc)
    return nc


def kernel(Q, K, V, mask, W_Q, W_K, W_V, W_O):
    import ml_dtypes
    from concourse.bass_utils import run_bass_kernel_spmd

    Q = np.asarray(Q, dtype=np.float32)
    K = np.asarray(K, dtype=np.float32)
    V = np.asarray(V, dtype=np.float32)
    W_Q = np.asarray(W_Q, dtype=np.float32)
    W_K = np.asarray(W_K, dtype=np.float32)
    W_V = np.asarray(W_V, dtype=np.float32)
    W_O = np.asarray(W_O, dtype=np.float32)
    mask = np.asarray(mask)

    nc = _build()

    # Additive mask: NEG where masked, 0 elsewhere (bf16; folded into scores
    # via an identity-matmul PSUM accumulate).
    mf = np.where(mask, np.float32(NEG), np.float32(0.0)).astype(ml_dtypes.bfloat16)

    in_maps = []
    for c in range(NCORES):
        b, j = c // 4, c % 4
        rows = slice(j * HD, (j + 1) * HD)
        in_maps.append(
            {
                "q_in": np.ascontiguousarray(Q[b]),
                "k_in": np.ascontiguousarray(K[b]),
                "v_in": np.ascontiguousarray(V[b]),
                "mask_in": mf,
                # scores scale 1/sqrt(DK) folded into W_Q
                "wqt": np.ascontiguousarray((W_Q[rows] / np.sqrt(DK)).T.astype(np.float32)),
                "wkt": np.ascontiguousarray(W_K[rows].T),
                "wvt": np.ascontiguousarray(W_V[rows].T),
                # eval-mode dropout scale 0.9 folded into W_O
                "wot": np.ascontiguousarray(W_O[:, rows].T * np.float32(0.9)),
            }
        )

    out = run_bass_kernel_spmd(nc, in_maps, core_ids=list(range(NCORES)), trace=TRACE)
    _LAST_PROFILE.clear()
    _LAST_PROFILE["exec_time_ns"] = out.exec_time_ns
    _LAST_PROFILE["mean_exec_time_ns"] = getattr(out, "mean_exec_time_ns", None)

    attn = np.empty((B, H, T, T), np.float32)
    y = np.zeros((B, T, D), np.float32)
    for c in range(NCORES):
        b, j = c // 4, c % 4
        r = out.results[c]
        attn[b, j * HPC:(j + 1) * HPC] = r["attn_out"]
        y[b] += np.asarray(r["y_out"])
    return y, attn


# revision 6
# speedup vs baseline: 1.0005x; 1.0005x over previous
"""Multi-head attention forward on 8 Trainium2 NeuronCores.

Sharding: core c handles batch b=c//4 and heads 4*(c%4) .. 4*(c%4)+3.
Tensor-parallel over heads (W_Q/W_K/W_V column-split, W_O row-split) plus
data-parallel over batch. Partial output projections are summed on host.

Problem constants (hardcoded): B=2, T=2048, D=1024, H=16, DK=64.
Returns (output [B,T,D], attn [B,H,T,T]) matching the reference module.
"""

import functools

import numpy as np

B, T, D, H = 2, 2048, 1024, 16
DK = D // H            # 64
HPC = 4                # heads per core
HD = HPC * DK          # 256 head-dims per core
NEG = -999999999.0
P = 128
NSTRIP = T // P        # 16
NCORES = 8

TRACE = False          # set by test harness for profiling runs
_LAST_PROFILE = {}


def _split_multi_waits(nc):
    """walrus in this toolchain rejects instructions with >1 sync wait;
    split extra waits onto preceding same-engine NoOps (engine queues are
    in-order, so a NoOp-wait before the instruction is equivalent)."""
    from concourse import mybir

    for func in nc.m.functions:
        for bb in func.blocks:
            new = []
            for inst in bb.instructions:
                si = inst.sync_info
                waits = list(si.on_wait) if si and si.on_wait else []
                if len(waits) > 1:
                    for j, w in enumerate(waits[:-1]):
                        new.append(
                            mybir.InstNoOp(
                                name=f"{inst.name}-w{j}",
                                engine=inst.engine,
                                sync_info=mybir.SyncInfo(on_wait=[w], on_update=[]),
                            )
                        )
                    si.on_wait = [waits[-1]]
                new.append(inst)
            bb.instructions = new


@functools.lru_cache(maxsize=1)
def _build():
    from contextlib import ExitStack

    import concourse.bass as bass
    import concourse.mybir as mybir
    import concourse.tile as tile
    import ml_dtypes

    f32 = mybir.dt.float32
    bf16 = mybir.dt.bfloat16
    Exp = mybir.ActivationFunctionType.Exp

    nc = bass.Bass()

    q_in = nc.dram_tensor("q_in", [T, D], f32, kind="ExternalInput")
    k_in = nc.dram_tensor("k_in", [T, D], f32, kind="ExternalInput")
    v_in = nc.dram_tensor("v_in", [T, D], f32, kind="ExternalInput")
    mask_in = nc.dram_tensor("mask_in", [T, T], bf16, kind="ExternalInput")
    wqt_in = nc.dram_tensor("wqt", [D, HD], f32, kind="ExternalInput")
    wkt_in = nc.dram_tensor("wkt", [D, HD], f32, kind="ExternalInput")
    wvt_in = nc.dram_tensor("wvt", [D, HD], f32, kind="ExternalInput")
    wot_in = nc.dram_tensor("wot", [HD, D], f32, kind="ExternalInput")
    attn_out = nc.dram_tensor("attn_out", [HPC, T, T], f32, kind="ExternalOutput")
    y_out = nc.dram_tensor("y_out", [T, D], f32, kind="ExternalOutput")

    ident_f_dram = nc.inline_tensor(np.eye(P, dtype=np.float32), "ident_f")
    ident_b_dram = nc.inline_tensor(
        np.eye(P, dtype=np.float32).astype(ml_dtypes.bfloat16), "ident_b"
    )

    with tile.TileContext(nc) as tc, ExitStack() as ctx:
        res = ctx.enter_context(tc.tile_pool(name="resident", bufs=1))

        # Weights: [D, HD] -> [128, D/128, HD]; chunk dc = rows dc*128..+128
        wqt_sb = res.tile([P, D // P, HD], f32)
        wkt_sb = res.tile([P, D // P, HD], f32)
        wvt_sb = res.tile([P, D // P, HD], f32)
        nc.sync.dma_start(out=wqt_sb, in_=wqt_in.rearrange("(c p) h -> p c h", p=P))
        nc.sync.dma_start(out=wkt_sb, in_=wkt_in.rearrange("(c p) h -> p c h", p=P))
        nc.sync.dma_start(out=wvt_sb, in_=wvt_in.rearrange("(c p) h -> p c h", p=P))
        # W_O^T: [HD, D] -> [128, 2, D]
        wot_sb = res.tile([P, HD // P, D], f32)
        nc.sync.dma_start(out=wot_sb, in_=wot_in.rearrange("(c p) h -> p c h", p=P))

        ident_f = res.tile([P, P], f32)
        ident_b = res.tile([P, P], bf16)
        nc.sync.dma_start(out=ident_f, in_=ident_f_dram[:, :])
        nc.sync.dma_start(out=ident_b, in_=ident_b_dram[:, :])

        # Resident activations
        qT_sb = res.tile([P, 2, T], f32)      # [head-dim 256 (2 blocks), T]
        kT_sb = res.tile([P, 2, T], f32)
        v_sb = res.tile([P, NSTRIP, HD], f32)  # v natural: block kb = rows kb*128..
        hT_sb = res.tile([P, 2, T], f32)       # heads^T (concat) [256, T]

        # ---- Phase 1: input transposes + projections ----
        with (
            tc.tile_pool(name="ph1_in", bufs=2) as ph1_in,
            tc.tile_pool(name="ph1_xt", bufs=2) as ph1_xt,
            tc.tile_pool(name="ph1_pxt", bufs=2, space="PSUM") as ph1_pxt,
            tc.tile_pool(name="ph1_po", bufs=4, space="PSUM") as ph1_po,
        ):
            for g in range(T // 512):  # groups of 512 tokens
                xts = {}
                for name, xdram in (("q", q_in), ("k", k_in), ("v", v_in)):
                    xnat = ph1_in.tile([P, 4, D], f32)
                    nc.sync.dma_start(
                        out=xnat,
                        in_=xdram[g * 512:(g + 1) * 512, :].rearrange(
                            "(a p) d -> p a d", p=P
                        ),
                    )
                    # transpose to xt4[p, dc, t] = X^T chunks [128d, 512t]
                    xt4 = ph1_xt.tile([P, D // P, 512], f32)
                    for tb in range(4):
                        ps = ph1_pxt.tile([P, D], f32)
                        for dc in range(D // P):
                            nc.tensor.transpose(
                                ps[:, dc * P:(dc + 1) * P],
                                xnat[:, tb, dc * P:(dc + 1) * P],
                                ident_f,
                            )
                        eng = nc.vector if tb % 2 == 0 else nc.scalar
                        if eng is nc.vector:
                            eng.tensor_copy(
                                xt4[:, :, tb * P:(tb + 1) * P],
                                ps.rearrange("p (c t) -> p c t", t=P),
                            )
                        else:
                            eng.copy(
                                xt4[:, :, tb * P:(tb + 1) * P],
                                ps.rearrange("p (c t) -> p c t", t=P),
                            )
                    xts[name] = xt4

                # q^T, k^T: out [HD(2x128 blocks), 512]
                for wsb, dst in ((wqt_sb, qT_sb), (wkt_sb, kT_sb)):
                    for m in range(2):
                        ps = ph1_po.tile([P, 512], f32)
                        for dc in range(D // P):
                            nc.tensor.matmul(
                                ps,
                                lhsT=wsb[:, dc, m * P:(m + 1) * P],
                                rhs=xts["q" if wsb is wqt_sb else "k"][:, dc, :],
                                start=(dc == 0),
                                stop=(dc == D // P - 1),
                            )
                        nc.scalar.copy(dst[:, m, g * 512:(g + 1) * 512], ps)

                # v natural: out [128 tokens, HD] per token-block
                for tb in range(4):
                    ps = ph1_po.tile([P, 512], f32)
                    for dc in range(D // P):
                        nc.tensor.matmul(
                            ps[:, :HD],
                            lhsT=xts["v"][:, dc, tb * P:(tb + 1) * P],
                            rhs=wvt_sb[:, dc, :],
                            start=(dc == 0),
                            stop=(dc == D // P - 1),
                        )
                    nc.vector.tensor_copy(v_sb[:, g * 4 + tb, :], ps[:, :HD])

        # ---- Phase 2: scores + softmax + attn out + AV (4-strip groups) ----
        # ---- with per-group output projection interleaved ----
        GS = 4  # strips per group
        with (
            tc.tile_pool(name="mk", bufs=5) as mk_pool,
            tc.tile_pool(name="expp", bufs=2) as exp_pool,
            tc.tile_pool(name="attnp", bufs=3) as attn_pool,
            tc.tile_pool(name="attnTp", bufs=1) as attnT_pool,
            tc.tile_pool(name="zp", bufs=16) as z_pool,
            tc.tile_pool(name="yp", bufs=2) as y_pool,
            tc.tile_pool(name="ps_s", bufs=2, space="PSUM") as ps_s,
            tc.tile_pool(name="ps_t", bufs=1, space="PSUM") as ps_t,
            tc.tile_pool(name="ps_av", bufs=2, space="PSUM") as ps_av,
        ):
            for g in range(NSTRIP // GS):
                mks = []
                for s in range(GS):
                    i = g * GS + s
                    mk = mk_pool.tile([P, T], bf16, tag="mk")
                    nc.sync.dma_start(out=mk, in_=mask_in[i * P:(i + 1) * P, :])
                    mks.append(mk)
                for h in range(HPC):
                    m, poff = h // 2, 64 * (h % 2)
                    # attnT for the whole group: [128, kb(16) x 512]
                    attnT = attnT_pool.tile([P, NSTRIP, GS * P], f32)
                    for s in range(GS):
                        i = g * GS + s
                        exp_sb = exp_pool.tile([P, T], f32, tag="exp")
                        zs = []
                        for half in range(2):
                            ps = ps_s.tile([P, 1024], f32, tag="s")
                            for ck in range(2):
                                sl = slice(ck * 512, (ck + 1) * 512)
                                ksl = slice(half * 1024 + ck * 512,
                                            half * 1024 + (ck + 1) * 512)
                                nc.tensor.matmul(
                                    ps[:, sl],
                                    lhsT=qT_sb[poff:poff + 64, m,
                                               i * P:(i + 1) * P],
                                    rhs=kT_sb[poff:poff + 64, m, ksl],
                                    start=True,
                                    stop=False,
                                )
                                nc.tensor.matmul(
                                    ps[:, sl],
                                    lhsT=ident_b,
                                    rhs=mks[s][:, ksl],
                                    start=False,
                                    stop=True,
                                )
                            zh = z_pool.tile([P, 1], f32, tag="z")
                            nc.scalar.activation(
                                exp_sb[:, half * 1024:(half + 1) * 1024],
                                ps,
                                Exp,
                                accum_out=zh,
                            )
                            zs.append(zh)
                        z = z_pool.tile([P, 1], f32, tag="z")
                        rec = z_pool.tile([P, 1], f32, tag="z")
                        nc.vector.tensor_add(z, zs[0], zs[1])
                        nc.vector.reciprocal(rec, z)
                        attn_sb = attn_pool.tile([P, T], f32, tag="attn")
                        nc.vector.tensor_scalar_mul(attn_sb, exp_sb, rec)
                        nc.sync.dma_start(
                            out=attn_out[h, i * P:(i + 1) * P, :], in_=attn_sb
                        )
                        # transpose attn strip into the group buffer
                        for half in range(2):
                            pst = ps_t.tile([P, 1024], f32, tag="t")
                            for r in range(8):
                                rr = half * 8 + r
                                nc.tensor.transpose(
                                    pst[:, r * P:(r + 1) * P],
                                    attn_sb[:, rr * P:(rr + 1) * P],
                                    ident_f,
                                )
                            dst = attnT[:, half * 8:(half + 1) * 8,
                                        s * P:(s + 1) * P]
                            src = pst.rearrange("p (c t) -> p c t", t=P)
                            if half == 0:
                                nc.scalar.copy(dst, src)
                            else:
                                nc.vector.tensor_copy(dst, src)
                    # AV over the group: heads^T [64, GS*128]
                    pav = ps_av.tile([64, GS * P], f32, tag="av")
                    for kb in range(NSTRIP):
                        nc.tensor.matmul(
                            pav,
                            lhsT=v_sb[:, kb, h * DK:(h + 1) * DK],
                            rhs=attnT[:, kb, :],
                            start=(kb == 0),
                            stop=(kb == NSTRIP - 1),
                        )
                    nc.vector.tensor_copy(
                        hT_sb[poff:poff + 64, m, g * GS * P:(g + 1) * GS * P],
                        pav,
                    )
                # output projection for this group's strips
                for s in range(GS):
                    i = g * GS + s
                    y_sb = y_pool.tile([P, D], f32, tag="y")
                    for oc in range(2):
                        ps = ps_t.tile([P, 512], f32, tag="t")
                        for m in range(2):
                            nc.tensor.matmul(
                                ps,
                                lhsT=hT_sb[:, m, i * P:(i + 1) * P],
                                rhs=wot_sb[:, m, oc * 512:(oc + 1) * 512],
                                start=(m == 0),
                                stop=(m == 1),
                            )
                        nc.vector.tensor_copy(y_sb[:, oc * 512:(oc + 1) * 512], ps)
                    nc.sync.dma_start(out=y_out[i * P:(i + 1) * P, :], in_=y_sb)

    _split_multi_waits(nc)
    return nc


def kernel(Q, K, V, mask, W_Q, W_K, W_V, W_O):
    import ml_dtypes
    from concourse.bass_utils import run_bass_kernel_spmd

    Q = np.asarray(Q, dtype=np.float32)
    K = np.asarray(K, dtype=np.float32)
    V = np.asarray(V, dtype=np.float32)
    W_Q = np.asarray(W_Q, dtype=np.float32)
    W_K = np.asarray(W_K, dtype=np.float32)
    W_V = np.asarray(W_V, dtype=np.float32)
    W_O = np.asarray(W_O, dtype=np.float32)
    mask = np.asarray(mask)

    nc = _build()

    # Additive mask: NEG where masked, 0 elsewhere (bf16; folded into scores
    # via an identity-matmul PSUM accumulate).
    mf = np.where(mask, np.float32(NEG), np.float32(0.0)).astype(ml_dtypes.bfloat16)

    in_maps = []
    for c in range(NCORES):
        b, j = c // 4, c % 4
        rows = slice(j * HD, (j + 1) * HD)
        in_maps.append(
            {
                "q_in": np.ascontiguousarray(Q[b]),
                "k_in": np.ascontiguousarray(K[b]),
                "v_in": np.ascontiguousarray(V[b]),
                "mask_in": mf,
                # scores scale 1/sqrt(DK) folded into W_Q
                "wqt": np.ascontiguousarray((W_Q[rows] / np.sqrt(DK)).T.astype(np.float32)),
                "wkt": np.ascontiguousarray(W_K[rows].T),
                "wvt": np.ascontiguousarray(W_V[rows].T),
                # eval-mode dropout scale 0.9 folded into W_O
                "wot": np.ascontiguousarray(W_O[:, rows].T * np.float32(0.9)),
            }
        )

    out = run_bass_kernel_spmd(nc, in_maps, core_ids=list(range(NCORES)), trace=TRACE)
    _LAST_PROFILE.clear()
    _LAST_PROFILE["exec_time_ns"] = out.exec_time_ns
    _LAST_PROFILE["mean_exec_time_ns"] = getattr(out, "mean_exec_time_ns", None)

    attn = np.empty((B, H, T, T), np.float32)
    y = np.zeros((B, T, D), np.float32)
    for c in range(NCORES):
        b, j = c // 4, c % 4
        r = out.results[c]
        attn[b, j * HPC:(j + 1) * HPC] = r["attn_out"]
        y[b] += np.asarray(r["y_out"])
    return y, attn
